# revision 14
# baseline (speedup 1.0000x reference)
"""Trainium2 Bass kernel for the InteractPre co-attention module.

Math (reference):
    p  = relu(protein @ Wc.T + bc)           [L, 256]
    r  = relu(reactions @ W2.T + b2)         [Q, 64]
    k  = relu(p @ W1.T + b1)                 [L, 64]
    ra = r @ Wra.T + bra                     [Q, 64]
    pa = k @ Wpa.T + bpa                     [L, 64]
    A  = relu(ra[:,None,:] + pa[None,:,:]) @ Wa.T + ba   [Q, L, 64]
    r_gate = sigmoid(mean_l A);  p_gate = sigmoid(mean_q A)
    rxnfp = r*(1+r_gate); prot = max_l k*(1+p_gate)
    out = MLP(concat([rxnfp, prot]))         [Q]

A is never materialized: only S_r[q] = sum_l relu(ra[q]+pa[l]) and
S_p[l] = sum_q relu(ra[q]+pa[l]) are needed (Wa matmul is linear).

Sharding: Q axis across 8 cores (64 reactions each) for the pairwise
stage, so each elementwise instruction spans free=L=4096.  The protein
conv stays L-sharded; pa/k are AllGathered after it.  S_r is then fully
local; only S_p (the PE "fold" of the pairwise tiles) needs a single
AllReduce, into which the (masked) ra column-sum correction rides.

Pairwise producer trick: relu(pa + ra) = max(pa, -ra) + ra, so one DVE
tensor_scalar (op0=max with -ra col, op1=add +ra col) emits the tile at
the 4x DVE rate; with accum_out it emits the *shifted* tile
max(pa,-ra) plus accum = sum_l max(pa,-ra) + ra = S_r - (L-1)*ra, fixed
up by a per-column linear correction.  The shifted tiles fold to
S_p - sum_{q in DVE tiles} ra, fixed up through the AllReduce payload.
ACT-produced tiles use plain Relu+bias with exact accumulation.  Tiles
destined for fp8 fold in pairs via DoubleRow matmuls (0.5 cyc/row).
"""

import os
import sys

import numpy as np

if "/opt/trn_rl_repo" not in sys.path:
    sys.path.insert(0, "/opt/trn_rl_repo")

Q = 512
L = 4096
NCORES = 8
Q_LOC = Q // NCORES          # 64 reactions per core
L_SH = L // NCORES           # 512 protein rows per core (conv shard)
NT = Q_LOC // 2              # 32 pairwise tiles (2 q's per tile)
D = 64                       # co-attention channel count
NCH = L // 512               # 8 fold chunks of 512

# --- tile type schedule ---------------------------------------------------
# 'B' = DVE bf16 (shifted-relu tensor_scalar), '8' = DVE fp8 (pair, DR fold),
# 'A' = ACT fp8 (plain relu, pair, DR fold).  Pairs must be adjacent.
_SCHED_DEFAULT = "B88BAAB88BAAB88BAAB88BAAB88BBBBB"
SCHED = os.environ.get("K_SCHED", _SCHED_DEFAULT)
assert len(SCHED) == NT

_CACHE = {}


def _pairs(sched):
    """Return list of (start_j, type) units: singles for B, pairs for 8/A."""
    units = []
    j = 0
    while j < NT:
        t = sched[j]
        if t == "B":
            units.append((j, 1, t))
            j += 1
        else:
            assert sched[j + 1] == t, f"unpaired {t} at {j}"
            units.append((j, 2, t))
            j += 2
    return units


def _build():
    import concourse.bass as bass
    import concourse.bacc as bacc
    import concourse.tile as tile
    from concourse import mybir

    f32 = mybir.dt.float32
    bf16 = mybir.dt.bfloat16
    fp8 = mybir.dt.float8e4
    AF = mybir.ActivationFunctionType
    ALU = mybir.AluOpType
    DR = mybir.MatmulPerfMode.DoubleRow

    units = _pairs(SCHED)

    nc = bacc.Bacc("TRN2", target_bir_lowering=False, debug=False,
                   num_devices=NCORES)

    def din(name, shape, dt=f32):
        return nc.dram_tensor(name, list(shape), dt, kind="ExternalInput").ap()

    # ---- external inputs (host-prepped, transposed: channels x tokens) ----
    protT = din("protT", [1024, L_SH], bf16)   # per-core protein shard^T
    reactT = din("reactT", [256, Q_LOC], bf16)  # per-core reaction shard^T
    WcT = din("WcT", [1024, 256], bf16)
    W1T = din("W1T", [256, D], bf16)
    W2T = din("W2T", [256, D], bf16)
    WaT = din("WaT", [D, D], bf16)
    WpaT = din("WpaT", [D, D], bf16)
    WraT = din("WraT", [D, D], bf16)
    Wf1aT = din("Wf1aT", [D, 256], bf16)       # Wf1[:, :64].T
    Wf1bT = din("Wf1bT", [D, 256], bf16)       # Wf1[:, 64:].T
    Wf2T = din("Wf2T", [256, 128], bf16)
    Wf3T = din("Wf3T", [128, 1], bf16)
    bc_d = din("bc", [256, 1])
    b1_d = din("b1", [D, 1])
    b2_d = din("b2", [D, 1])
    ba_d = din("ba", [D, 1])
    bpa_d = din("bpa", [D, 1])
    bra_d = din("bra", [D, 1])
    nbra_d = din("nbra", [D, 1])               # -bra
    bf1_d = din("bf1", [256, 1])
    bf2_d = din("bf2", [128, 1])
    bf3_d = din("bf3", [1, 1])
    ifoldb_d = din("Ifoldb", [128, D], bf16)   # [I64; I64]
    ifold8_d = din("Ifold8", [128, 2 * D], fp8)  # [I;I | I;I] for DoubleRow
    maskL2_d = din("maskL2", [128, NT])        # (L-1) at DVE cols else 0
    maskD2_d = din("maskD2", [128, NT])        # 1 at DVE cols else 0

    out_d = nc.dram_tensor("out", [1, Q_LOC], f32, kind="ExternalOutput").ap()
    DEBUG = os.environ.get("K_DEBUG", "0") == "1"
    if DEBUG:
        dbg_prot = nc.dram_tensor("dbg_prot", [D, 1], f32,
                                  kind="ExternalOutput").ap()
        dbg_fsum = nc.dram_tensor("dbg_fsum", [D, L], f32,
                                  kind="ExternalOutput").ap()
        dbg_sr = nc.dram_tensor("dbg_sr", [D, Q_LOC], f32,
                                kind="ExternalOutput").ap()
        dbg_pg = nc.dram_tensor("dbg_pg", [D, L], f32,
                                kind="ExternalOutput").ap()
        dbg_sumra = nc.dram_tensor("dbg_sumra", [D, 1], f32,
                                   kind="ExternalOutput").ap()
        dbg_fsb = nc.dram_tensor("dbg_fsb", [128, 2049], f32,
                                 kind="ExternalOutput").ap()

    with tile.TileContext(nc) as tc:
        with (
            tc.tile_pool(name="const", bufs=1) as cp,
            tc.tile_pool(name="work", bufs=1) as wp,
            tc.tile_pool(name="tmpb", bufs=3) as tpb,
            tc.tile_pool(name="tmp8", bufs=2) as tp8,
            tc.tile_pool(name="psum", bufs=1, space="PSUM") as ps,
            tc.tile_pool(name="dram", bufs=1, space="DRAM") as dp,
        ):
            dma = nc.sync.dma_start

            def cload(src, shape, dt=f32, tag=None):
                t = cp.tile(list(shape), dt, tag=tag or src.tensor.name)
                dma(t[:], src)
                return t

            # ---------------- constants ----------------
            protT_sb, WcT_sb = [], []
            for i in range(8):
                protT_sb.append(cload(protT[i * 128:(i + 1) * 128, :],
                                      [128, L_SH], bf16, tag=f"protT{i}"))
                WcT_sb.append(cload(WcT[i * 128:(i + 1) * 128, :],
                                    [128, 256], bf16, tag=f"WcT{i}"))
            reactT_sb = [cload(reactT[i * 128:(i + 1) * 128, :],
                               [128, Q_LOC], bf16, tag=f"reactT{i}")
                         for i in range(2)]
            W1T_sb = [cload(W1T[i * 128:(i + 1) * 128, :], [128, D], bf16,
                            tag=f"W1T{i}") for i in range(2)]
            W2T_sb = [cload(W2T[i * 128:(i + 1) * 128, :], [128, D], bf16,
                            tag=f"W2T{i}") for i in range(2)]
            WaT_sb = cload(WaT, [D, D], bf16)
            WpaT_sb = cload(WpaT, [D, D], bf16)
            WraT_sb = cload(WraT, [D, D], bf16)
            Wf1aT_sb = cload(Wf1aT, [D, 256], bf16)
            Wf1bT_sb = cload(Wf1bT, [D, 256], bf16)
            Wf2T_sb = [cload(Wf2T[i * 128:(i + 1) * 128, :], [128, 128], bf16,
                             tag=f"Wf2T{i}") for i in range(2)]
            Wf3T_sb = cload(Wf3T, [128, 1], bf16)
            bc_sb = [cload(bc_d[i * 128:(i + 1) * 128, :], [128, 1],
                           tag=f"bc{i}") for i in range(2)]
            b1_sb = cload(b1_d, [D, 1])
            b2_sb = cload(b2_d, [D, 1])
            ba_sb = cload(ba_d, [D, 1])
            bpa_sb = cload(bpa_d, [D, 1])
            bra_sb = cload(bra_d, [D, 1])
            nbra_sb = cload(nbra_d, [D, 1])
            bf1_sb = [cload(bf1_d[i * 128:(i + 1) * 128, :], [128, 1],
                            tag=f"bf1{i}") for i in range(2)]
            bf2_sb = cload(bf2_d, [128, 1])
            bf3_sb = cload(bf3_d, [1, 1])
            ifoldb_sb = cload(ifoldb_d, [128, D], bf16)
            ifold8_sb = cp.tile([128, 2, D], fp8, tag="Ifold8")
            dma(ifold8_sb[:], ifold8_d)
            maskL2_sb = cload(maskL2_d, [128, NT])
            maskD2_sb = cload(maskD2_d, [128, NT])

            # one PSUM arena, manually laid out
            F = ps.tile([128, 4096], f32, tag="F")

            # ---------------- protein conv (L-sharded) ----------------
            p_sb = []
            for m in range(2):
                pslice = F[:, m * 512:(m + 1) * 512]
                for i in range(8):
                    nc.tensor.matmul(
                        pslice, WcT_sb[i][:, m * 128:(m + 1) * 128],
                        protT_sb[i][:], start=(i == 0), stop=(i == 7))
                pt = wp.tile([128, L_SH], bf16, tag=f"p{m}")
                nc.scalar.activation(pt[:], pslice, AF.Relu, bias=bc_sb[m][:])
                p_sb.append(pt)

            psk = F[0:D, 1024:1536]
            nc.tensor.matmul(psk, W1T_sb[0][:], p_sb[0][:],
                             start=True, stop=False)
            nc.tensor.matmul(psk, W1T_sb[1][:], p_sb[1][:],
                             start=False, stop=True)
            # stack [k ; pa] for the AllGather (k on top: matmul rhs needs
            # base partition 0)
            ag_sb = wp.tile([128, L_SH], bf16, tag="ag")
            nc.scalar.activation(ag_sb[0:D, :], psk, AF.Relu, bias=b1_sb[:])

            pspa = F[0:D, 1536:2048]
            nc.tensor.matmul(pspa, WpaT_sb[:], ag_sb[0:D, :],
                             start=True, stop=True)
            # evac at base 0, then DMA into the upper half (ACT cannot
            # shift partitions; DMA can)
            patmp = wp.tile([D, L_SH], bf16, tag="patmp")
            nc.scalar.activation(patmp[:], pspa, AF.Identity, bias=bpa_sb[:])
            dma(ag_sb[D:128, :], patmp[:])

            # ---------------- AllGather pa/k ----------------
            cc_ag_in = dp.tile([128, L_SH], bf16)
            cc_ag_out = dp.tile([NCORES, 128, L_SH], bf16, addr_space="Shared")
            dma(cc_ag_in[:], ag_sb[:])
            nc.gpsimd.collective_compute(
                "AllGather", mybir.AluOpType.bypass,
                replica_groups=[list(range(NCORES))],
                ins=[cc_ag_in[:].opt()],
                outs=[cc_ag_out[:].opt()],
            )

            # ---------------- reaction side (local, overlaps AG) --------
            psr = F[0:D, 2048:2048 + Q_LOC]
            nc.tensor.matmul(psr, W2T_sb[0][:], reactT_sb[0][:],
                             start=True, stop=False)
            nc.tensor.matmul(psr, W2T_sb[1][:], reactT_sb[1][:],
                             start=False, stop=True)
            r_sb = wp.tile([D, Q_LOC], bf16, tag="r")
            nc.scalar.activation(r_sb[:], psr, AF.Relu, bias=b2_sb[:])

            psra = F[0:D, 2112:2112 + Q_LOC]
            nc.tensor.matmul(psra, WraT_sb[:], r_sb[:], start=True, stop=True)
            ra_sb = wp.tile([D, Q_LOC], f32, tag="ra")    # +ra
            nra_sb = wp.tile([D, Q_LOC], f32, tag="nra")  # -ra
            nc.scalar.activation(ra_sb[:], psra, AF.Identity, bias=bra_sb[:])
            nc.scalar.activation(nra_sb[:], psra, AF.Identity,
                                 bias=nbra_sb[:], scale=-1.0)

            # q-pair layout: col j <-> q=j (rows 0:64), q=j+32 (rows 64:128)
            ra2 = wp.tile([128, NT], f32, tag="ra2")
            nra2 = wp.tile([128, NT], f32, tag="nra2")
            dma(ra2[0:D, :], ra_sb[:, 0:NT])
            dma(ra2[D:128, :], ra_sb[:, NT:Q_LOC])
            dma(nra2[0:D, :], nra_sb[:, 0:NT])
            dma(nra2[D:128, :], nra_sb[:, NT:Q_LOC])

            # pa2: pa duplicated on both partition halves; k_full separate
            # (AG output is [core, chan, x]; transpose to [chan, core, x])
            pa2 = wp.tile([128, L], bf16, tag="pa2")
            k_sb = wp.tile([D, L], bf16, tag="k")
            ag_k = cc_ag_out[:, 0:D, :].transpose([1, 0, 2])
            ag_pa = cc_ag_out[:, D:128, :].transpose([1, 0, 2])
            dma(pa2[0:D, :], ag_pa)
            dma(pa2[D:128, :], ag_pa)
            dma(k_sb[:], ag_k)

            # ---------------- pairwise stage ----------------
            acc2 = wp.tile([128, NT], f32, tag="acc2")

            def fold(rhs_ap, first, last, dr):
                for ch in range(NCH):
                    if dr:
                        nc.tensor.matmul(
                            F[0:D, ch * 512:(ch + 1) * 512],
                            ifold8_sb[:, :, :],
                            rhs_ap[:, :, ch * 512:(ch + 1) * 512],
                            start=first, stop=last, perf_mode=DR)
                    else:
                        nc.tensor.matmul(
                            F[0:D, ch * 512:(ch + 1) * 512],
                            ifoldb_sb[:],
                            rhs_ap[:, ch * 512:(ch + 1) * 512],
                            start=first, stop=last)

            nunits = len(units)
            for ui, (j0, w, t) in enumerate(units):
                first, last = (ui == 0), (ui == nunits - 1)
                if t == "B":
                    tmp = tpb.tile([128, L], bf16, tag="tb")
                    nc.vector.tensor_scalar(
                        tmp[:], pa2[:], nra2[:, j0:j0 + 1], ra2[:, j0:j0 + 1],
                        ALU.max, ALU.add, accum_out=acc2[:, j0:j0 + 1])
                    fold(tmp, first, last, dr=False)
                elif t == "8":
                    pair = tp8.tile([128, 2, L], fp8, tag="t8")
                    for s in range(2):
                        j = j0 + s
                        nc.vector.tensor_scalar(
                            pair[:, s, :], pa2[:], nra2[:, j:j + 1],
                            ra2[:, j:j + 1], ALU.max, ALU.add,
                            accum_out=acc2[:, j:j + 1])
                    fold(pair, first, last, dr=True)
                else:  # 'A'
                    pair = tp8.tile([128, 2, L], fp8, tag="tA")
                    for s in range(2):
                        j = j0 + s
                        nc.scalar.activation(
                            pair[:, s, :], pa2[:], AF.Relu,
                            bias=ra2[:, j:j + 1],
                            accum_out=acc2[:, j:j + 1])
                    fold(pair, first, last, dr=True)

            # masked ra column-sum for the S_p correction (rides the AR)
            rad = wp.tile([128, NT], f32, tag="rad")
            nc.vector.tensor_tensor(rad[:], ra2[:], maskD2_sb[:], op=ALU.mult)
            sumra2 = wp.tile([128, 1], f32, tag="sumra2")
            nc.vector.reduce_sum(sumra2[:], rad[:], axis=mybir.AxisListType.X)

            # ---------------- F evac + AllReduce ----------------
            # NOTE: the whole F[0:64, :] region belongs to the fold until
            # these evacs have read it; all later psum users come after.
            fsb = wp.tile([128, 2048 + 1], bf16, tag="fsb")
            nc.scalar.activation(fsb[0:D, 0:2048], F[0:D, 0:2048], AF.Copy)
            # DVE 64-part op may shift partition halves (quadrant-aligned)
            nc.vector.tensor_scalar(fsb[D:128, 0:2048], F[0:D, 2048:4096],
                                    0.0, None, ALU.add)
            nc.vector.tensor_scalar(fsb[:, 2048:2049], sumra2[:], 0.0, None,
                                    ALU.add)

            cc_ar_in = dp.tile([128, 2049], bf16)
            cc_ar_out = dp.tile([128, 2049], bf16, addr_space="Shared")
            dma(cc_ar_in[:], fsb[:])
            nc.gpsimd.collective_compute(
                "AllReduce", mybir.AluOpType.add,
                replica_groups=[list(range(NCORES))],
                ins=[cc_ar_in[:].opt()],
                outs=[cc_ar_out[:].opt()],
            )

            # ---------------- S_r (local) -> rxnfp; overlaps the AR ------
            corr = wp.tile([128, NT], f32, tag="corr")
            nc.vector.tensor_tensor(corr[:], ra2[:], maskL2_sb[:],
                                    op=ALU.mult)
            sr2 = wp.tile([128, NT], bf16, tag="sr2")
            nc.vector.tensor_tensor(sr2[:], acc2[:], corr[:], op=ALU.add)
            sr = wp.tile([D, Q_LOC], bf16, tag="sr")
            dma(sr[:, 0:NT], sr2[0:D, :])
            dma(sr[:, NT:Q_LOC], sr2[D:128, :])

            psrg = F[0:D, 2176:2176 + Q_LOC]
            nc.tensor.matmul(psrg, WaT_sb[:], sr[:], start=True, stop=True)
            rgate = wp.tile([D, Q_LOC], f32, tag="rgate")
            nc.scalar.activation(rgate[:], psrg, AF.Sigmoid, bias=ba_sb[:],
                                 scale=1.0 / L)
            rx = wp.tile([D, Q_LOC], bf16, tag="rx")
            nc.vector.scalar_tensor_tensor(rx[:], rgate[:], 1.0, r_sb[:],
                                           op0=ALU.add, op1=ALU.mult)

            fsum = wp.tile([D, L], bf16, tag="fsum")
            dma(fsum[:, 0:2048], cc_ar_out[0:D, 0:2048])
            dma(fsum[:, 2048:4096], cc_ar_out[D:128, 0:2048])
            sra_t = wp.tile([D, 1], bf16, tag="sra_t")
            sra_b = wp.tile([D, 1], bf16, tag="sra_b")
            dma(sra_t[:], cc_ar_out[0:D, 2048:2049])
            dma(sra_b[:], cc_ar_out[D:128, 2048:2049])
            sumra_g = wp.tile([D, 1], bf16, tag="sumra_g")
            nc.vector.tensor_tensor(sumra_g[:], sra_t[:], sra_b[:],
                                    op=ALU.add)

            # p_gate bias col: ba + Wa @ sumra_glob / Q
            psbb = F[0:D, 2304:2305]
            nc.tensor.matmul(psbb, WaT_sb[:], sumra_g[:],
                             start=True, stop=True)
            babar = wp.tile([D, 1], f32, tag="babar")
            nc.scalar.activation(babar[:], psbb, AF.Identity, bias=ba_sb[:],
                                 scale=1.0 / Q)

            # ---------------- p_gate / prot (replicated) ----------------
            pgate = wp.tile([D, L], bf16, tag="pgate")
            for ch in range(NCH):
                pspg = F[0:D, ch * 512:(ch + 1) * 512]
                nc.tensor.matmul(pspg, WaT_sb[:],
                                 fsum[:, ch * 512:(ch + 1) * 512],
                                 start=True, stop=True)
                nc.scalar.activation(pgate[:, ch * 512:(ch + 1) * 512], pspg,
                                     AF.Sigmoid, bias=babar[:], scale=1.0 / Q)
            g_sb = wp.tile([D, L], bf16, tag="g")
            nc.vector.scalar_tensor_tensor(g_sb[:], pgate[:], 1.0, k_sb[:],
                                           op0=ALU.add, op1=ALU.mult)
            protg = wp.tile([D, 1], bf16, tag="protg")
            nc.vector.reduce_max(protg[:], g_sb[:], axis=mybir.AxisListType.X)

            # ---------------- MLP head (local q) ----------------
            h1_sb = []
            for m in range(2):
                pst = F[0:128, 2432 + m * 2:2432 + m * 2 + 1]
                nc.tensor.matmul(pst, Wf1bT_sb[:, m * 128:(m + 1) * 128],
                                 protg[:], start=True, stop=True)
                fold_sb = wp.tile([128, 1], f32, tag=f"fold{m}")
                nc.scalar.activation(fold_sb[:], pst, AF.Identity,
                                     bias=bf1_sb[m][:])
                psh = F[:, 2560 + m * 64:2560 + (m + 1) * 64]
                nc.tensor.matmul(psh, Wf1aT_sb[:, m * 128:(m + 1) * 128],
                                 rx[:], start=True, stop=True)
                h1l = wp.tile([128, Q_LOC], f32, tag=f"h1l{m}")
                nc.scalar.activation(h1l[:], psh, AF.Identity,
                                     bias=fold_sb[:])
                h1 = wp.tile([128, Q_LOC], bf16, tag=f"h1{m}")
                nc.vector.scalar_tensor_tensor(h1[:], h1l[:], 0.01, h1l[:],
                                               op0=ALU.mult, op1=ALU.max)
                h1_sb.append(h1)

            psh2 = F[:, 2816:2816 + Q_LOC]
            nc.tensor.matmul(psh2, Wf2T_sb[0][:], h1_sb[0][:],
                             start=True, stop=False)
            nc.tensor.matmul(psh2, Wf2T_sb[1][:], h1_sb[1][:],
                             start=False, stop=True)
            h2l = wp.tile([128, Q_LOC], f32, tag="h2l")
            nc.scalar.activation(h2l[:], psh2, AF.Identity, bias=bf2_sb[:])
            h2 = wp.tile([128, Q_LOC], bf16, tag="h2")
            nc.vector.scalar_tensor_tensor(h2[:], h2l[:], 0.01, h2l[:],
                                           op0=ALU.mult, op1=ALU.max)

            if DEBUG:
                dbgfs = wp.tile([128, 2049], f32, tag="dbgfs")
                nc.vector.tensor_scalar(dbgfs[:], fsb[:], 0.0, None, ALU.add)
                dma(dbg_fsb, dbgfs[:])
                dbgp = wp.tile([D, 1], f32, tag="dbgp")
                nc.vector.tensor_scalar(dbgp[:], protg[:], 0.0, None, ALU.add)
                dma(dbg_prot, dbgp[:])
                dbgf = wp.tile([D, L], f32, tag="dbgf")
                nc.vector.tensor_scalar(dbgf[:], fsum[:], 0.0, None, ALU.add)
                dma(dbg_fsum, dbgf[:])
                dbgs = wp.tile([D, Q_LOC], f32, tag="dbgs")
                nc.vector.tensor_scalar(dbgs[:], sr[:], 0.0, None, ALU.add)
                dma(dbg_sr, dbgs[:])
                dbgg = wp.tile([D, L], f32, tag="dbgg")
                nc.vector.tensor_scalar(dbgg[:], pgate[:], 0.0, None, ALU.add)
                dma(dbg_pg, dbgg[:])
                dbgsr = wp.tile([D, 1], f32, tag="dbgsr")
                nc.vector.tensor_scalar(dbgsr[:], sumra_g[:], 0.0, None,
                                        ALU.add)
                dma(dbg_sumra, dbgsr[:])

            pso = F[0:1, 3072:3072 + Q_LOC]
            nc.tensor.matmul(pso, Wf3T_sb[:], h2[:], start=True, stop=True)
            out_sb = wp.tile([1, Q_LOC], f32, tag="out")
            nc.scalar.activation(out_sb[:], pso, AF.Identity, bias=bf3_sb[:])
            dma(out_d, out_sb[:])

    nc.compile()
    return nc


def _get_nc():
    key = (SCHED,)
    if key not in _CACHE:
        _CACHE[key] = _build()
    return _CACHE[key]


def _prep_in_maps(inputs):
    from concourse import mybir
    bf16_np = mybir.dt.np(mybir.dt.bfloat16)
    fp8_np = mybir.dt.np(mybir.dt.float8e4)

    f = lambda x: np.ascontiguousarray(np.asarray(x), dtype=np.float32)
    protein = f(inputs["protein"])[0]          # [L, 1024]
    reactions = f(inputs["reactions"])[0]      # [Q, 256]
    Wc, bc = f(inputs["Wc"]), f(inputs["bc"])
    W1, b1 = f(inputs["W1"]), f(inputs["b1"])
    W2, b2 = f(inputs["W2"]), f(inputs["b2"])
    Wa, ba = f(inputs["Wa"]), f(inputs["ba"])
    Wpa, bpa = f(inputs["Wpa"]), f(inputs["bpa"])
    Wra, bra = f(inputs["Wra"]), f(inputs["bra"])
    Wf1, bf1 = f(inputs["Wf1"]), f(inputs["bf1"])
    Wf2, bf2 = f(inputs["Wf2"]), f(inputs["bf2"])
    Wf3, bf3 = f(inputs["Wf3"]), f(inputs["bf3"])

    c = np.ascontiguousarray
    eye = np.eye(D, dtype=np.float32)
    ifoldb = np.concatenate([eye, eye], axis=0)           # [128, 64]
    ifold8 = np.concatenate([ifoldb, ifoldb], axis=1)     # [128, 128]

    is_dve = np.array([1.0 if t in "B8" else 0.0 for t in SCHED],
                      np.float32)
    maskL2 = np.tile(is_dve * (L - 1), (128, 1)).astype(np.float32)
    maskD2 = np.tile(is_dve, (128, 1)).astype(np.float32)

    common = {
        "WcT": c(Wc.T).astype(bf16_np),
        "W1T": c(W1.T).astype(bf16_np),
        "W2T": c(W2.T).astype(bf16_np),
        "WaT": c(Wa.T).astype(bf16_np),
        "WpaT": c(Wpa.T).astype(bf16_np),
        "WraT": c(Wra.T).astype(bf16_np),
        "Wf1aT": c(Wf1[:, :D].T).astype(bf16_np),
        "Wf1bT": c(Wf1[:, D:].T).astype(bf16_np),
        "Wf2T": c(Wf2.T).astype(bf16_np),
        "Wf3T": c(Wf3.T).astype(bf16_np),
        "bc": bc.reshape(-1, 1),
        "b1": b1.reshape(-1, 1),
        "b2": b2.reshape(-1, 1),
        "ba": ba.reshape(-1, 1),
        "bpa": bpa.reshape(-1, 1),
        "bra": bra.reshape(-1, 1),
        "nbra": (-bra).reshape(-1, 1),
        "bf1": bf1.reshape(-1, 1),
        "bf2": bf2.reshape(-1, 1),
        "bf3": bf3.reshape(-1, 1),
        "Ifoldb": ifoldb.astype(bf16_np),
        "Ifold8": ifold8.astype(fp8_np),
        "maskL2": maskL2,
        "maskD2": maskD2,
    }
    reactT = c(reactions.T).astype(bf16_np)    # [256, 512]
    in_maps = []
    for d in range(NCORES):
        shard = c(protein[d * L_SH:(d + 1) * L_SH, :].T).astype(bf16_np)
        in_maps.append({**common, "protT": shard,
                        "reactT": c(reactT[:, d * Q_LOC:(d + 1) * Q_LOC])})
    return in_maps


def run(inputs, trace=False, **kw):
    from concourse import bass_utils
    nc = _get_nc()
    in_maps = _prep_in_maps(inputs)
    res = bass_utils.run_bass_kernel_spmd(
        nc, in_maps, core_ids=list(range(NCORES)), trace=trace, **kw)
    return res


def kernel(**inputs):
    res = run(inputs)
    return np.concatenate(
        [np.asarray(res.results[d]["out"], np.float32).reshape(-1)
         for d in range(NCORES)])


# revision 17
# speedup vs baseline: 1.2750x; 1.2750x over previous
"""Trainium2 Bass kernel for the InteractPre co-attention module.

Math (reference):
    p  = relu(protein @ Wc.T + bc)           [L, 256]
    r  = relu(reactions @ W2.T + b2)         [Q, 64]
    k  = relu(p @ W1.T + b1)                 [L, 64]
    ra = r @ Wra.T + bra                     [Q, 64]
    pa = k @ Wpa.T + bpa                     [L, 64]
    A  = relu(ra[:,None,:] + pa[None,:,:]) @ Wa.T + ba   [Q, L, 64]
    r_gate = sigmoid(mean_l A);  p_gate = sigmoid(mean_q A)
    rxnfp = r*(1+r_gate); prot = max_l k*(1+p_gate)
    out = MLP(concat([rxnfp, prot]))         [Q]

A is never materialized: only S_r[q] = sum_l relu(ra[q]+pa[l]) and
S_p[l] = sum_q relu(ra[q]+pa[l]) are needed (Wa matmul is linear).

Sharding: Q axis across 8 cores (64 reactions each) for the pairwise
stage, so each elementwise instruction spans free=L=4096.  The protein
conv stays L-sharded; pa/k are AllGathered after it.  S_r is then fully
local; only S_p (the PE "fold" of the pairwise tiles) needs a single
AllReduce, into which the (masked) ra column-sum correction rides.

Pairwise producer trick: relu(pa + ra) = max(pa, -ra) + ra, so one DVE
tensor_scalar (op0=max with -ra col, op1=add +ra col) emits the tile at
the 4x DVE rate; with accum_out it emits the *shifted* tile
max(pa,-ra) plus accum = sum_l max(pa,-ra) + ra = S_r - (L-1)*ra, fixed
up by a per-column linear correction.  The shifted tiles fold to
S_p - sum_{q in DVE tiles} ra, fixed up through the AllReduce payload.
ACT-produced tiles use plain Relu+bias with exact accumulation.  Tiles
destined for fp8 fold in pairs via DoubleRow matmuls (0.5 cyc/row).
"""

import os
import sys

import numpy as np

if "/opt/trn_rl_repo" not in sys.path:
    sys.path.insert(0, "/opt/trn_rl_repo")

Q = 512
L = 4096
NCORES = 8
Q_LOC = Q // NCORES          # 64 reactions per core
L_SH = L // NCORES           # 512 protein rows per core (conv shard)
NT = Q_LOC // 2              # 32 pairwise tiles (2 q's per tile)
D = 64                       # co-attention channel count
NCH = L // 512               # 8 fold chunks of 512

# --- tile type schedule ---------------------------------------------------
# 'B' = DVE bf16 (shifted-relu tensor_scalar), '8' = DVE fp8 (pair, DR fold),
# 'A' = ACT fp8 (plain relu, pair, DR fold).  Pairs must be adjacent.
# Measured HW rates: ACT produce+accum 3708ns/tile, DVE CACHE_REDUCE bf16
# 4533 / fp8 6705, PE fold bf16 3032 vs DR 1516 -> ACT-heavy split.
_SCHED_DEFAULT = "AABAABAA88BAABAABAA88BAABAABAABB"
SCHED = os.environ.get("K_SCHED", _SCHED_DEFAULT)
assert len(SCHED) == NT

_CACHE = {}


def _pairs(sched):
    """Return list of (start_j, type) units: singles for B, pairs for 8/A."""
    units = []
    j = 0
    while j < NT:
        t = sched[j]
        if t == "B":
            units.append((j, 1, t))
            j += 1
        else:
            assert sched[j + 1] == t, f"unpaired {t} at {j}"
            units.append((j, 2, t))
            j += 2
    return units


def _build():
    import concourse.bass as bass
    import concourse.bacc as bacc
    import concourse.tile as tile
    from concourse import mybir

    f32 = mybir.dt.float32
    bf16 = mybir.dt.bfloat16
    fp8 = mybir.dt.float8e4
    AF = mybir.ActivationFunctionType
    ALU = mybir.AluOpType
    DR = mybir.MatmulPerfMode.DoubleRow

    units = _pairs(SCHED)

    nc = bacc.Bacc("TRN2", target_bir_lowering=False, debug=False,
                   num_devices=NCORES)

    def din(name, shape, dt=f32):
        return nc.dram_tensor(name, list(shape), dt, kind="ExternalInput").ap()

    # ---- external inputs (host-prepped, transposed: channels x tokens) ----
    protT = din("protT", [1024, L_SH], bf16)   # per-core protein shard^T
    reactT = din("reactT", [256, Q_LOC], bf16)  # per-core reaction shard^T
    WcT = din("WcT", [1024, 256], bf16)
    W1T = din("W1T", [256, D], bf16)
    W2T = din("W2T", [256, D], bf16)
    WaT = din("WaT", [D, D], bf16)
    WpaT = din("WpaT", [D, D], bf16)
    WraT = din("WraT", [D, D], bf16)
    Wf1aT = din("Wf1aT", [D, 256], bf16)       # Wf1[:, :64].T
    Wf1bT = din("Wf1bT", [D, 256], bf16)       # Wf1[:, 64:].T
    Wf2T = din("Wf2T", [256, 128], bf16)
    Wf3T = din("Wf3T", [128, 1], bf16)
    bc_d = din("bc", [256, 1])
    b1_d = din("b1", [D, 1])
    b2_d = din("b2", [D, 1])
    ba_d = din("ba", [D, 1])
    bpa_d = din("bpa", [D, 1])
    bra_d = din("bra", [D, 1])
    nbra_d = din("nbra", [D, 1])               # -bra
    bf1_d = din("bf1", [256, 1])
    bf2_d = din("bf2", [128, 1])
    bf3_d = din("bf3", [1, 1])
    ifoldb_d = din("Ifoldb", [128, D], bf16)   # [I64; I64]
    ifold8_d = din("Ifold8", [128, 2 * D], fp8)  # [I;I | I;I] for DoubleRow
    maskL2_d = din("maskL2", [128, NT])        # (L-1) at DVE cols else 0
    maskD2_d = din("maskD2", [128, NT])        # 1 at DVE cols else 0

    out_d = nc.dram_tensor("out", [1, Q_LOC], f32, kind="ExternalOutput").ap()
    DEBUG = os.environ.get("K_DEBUG", "0") == "1"
    if DEBUG:
        dbg_prot = nc.dram_tensor("dbg_prot", [D, 1], f32,
                                  kind="ExternalOutput").ap()
        dbg_fsum = nc.dram_tensor("dbg_fsum", [D, L], f32,
                                  kind="ExternalOutput").ap()
        dbg_sr = nc.dram_tensor("dbg_sr", [D, Q_LOC], f32,
                                kind="ExternalOutput").ap()
        dbg_pg = nc.dram_tensor("dbg_pg", [D, L], f32,
                                kind="ExternalOutput").ap()
        dbg_sumra = nc.dram_tensor("dbg_sumra", [D, 1], f32,
                                   kind="ExternalOutput").ap()
        dbg_fsb = nc.dram_tensor("dbg_fsb", [128, 2049], f32,
                                 kind="ExternalOutput").ap()

    with tile.TileContext(nc) as tc:
        with (
            tc.tile_pool(name="const", bufs=1) as cp,
            tc.tile_pool(name="work", bufs=1) as wp,
            tc.tile_pool(name="tmpb", bufs=3) as tpb,
            tc.tile_pool(name="tmp8", bufs=2) as tp8,
            tc.tile_pool(name="psum", bufs=1, space="PSUM") as ps,
            tc.tile_pool(name="dram", bufs=1, space="DRAM") as dp,
        ):
            dma = nc.sync.dma_start

            def cload(src, shape, dt=f32, tag=None):
                t = cp.tile(list(shape), dt, tag=tag or src.tensor.name)
                dma(t[:], src)
                return t

            # ---------------- constants ----------------
            protT_sb, WcT_sb = [], []
            for i in range(8):
                protT_sb.append(cload(protT[i * 128:(i + 1) * 128, :],
                                      [128, L_SH], bf16, tag=f"protT{i}"))
                WcT_sb.append(cload(WcT[i * 128:(i + 1) * 128, :],
                                    [128, 256], bf16, tag=f"WcT{i}"))
            reactT_sb = [cload(reactT[i * 128:(i + 1) * 128, :],
                               [128, Q_LOC], bf16, tag=f"reactT{i}")
                         for i in range(2)]
            W1T_sb = [cload(W1T[i * 128:(i + 1) * 128, :], [128, D], bf16,
                            tag=f"W1T{i}") for i in range(2)]
            W2T_sb = [cload(W2T[i * 128:(i + 1) * 128, :], [128, D], bf16,
                            tag=f"W2T{i}") for i in range(2)]
            WaT_sb = cload(WaT, [D, D], bf16)
            WpaT_sb = cload(WpaT, [D, D], bf16)
            WraT_sb = cload(WraT, [D, D], bf16)
            Wf1aT_sb = cload(Wf1aT, [D, 256], bf16)
            Wf1bT_sb = cload(Wf1bT, [D, 256], bf16)
            Wf2T_sb = [cload(Wf2T[i * 128:(i + 1) * 128, :], [128, 128], bf16,
                             tag=f"Wf2T{i}") for i in range(2)]
            Wf3T_sb = cload(Wf3T, [128, 1], bf16)
            bc_sb = [cload(bc_d[i * 128:(i + 1) * 128, :], [128, 1],
                           tag=f"bc{i}") for i in range(2)]
            b1_sb = cload(b1_d, [D, 1])
            b2_sb = cload(b2_d, [D, 1])
            ba_sb = cload(ba_d, [D, 1])
            bpa_sb = cload(bpa_d, [D, 1])
            bra_sb = cload(bra_d, [D, 1])
            nbra_sb = cload(nbra_d, [D, 1])
            bf1_sb = [cload(bf1_d[i * 128:(i + 1) * 128, :], [128, 1],
                            tag=f"bf1{i}") for i in range(2)]
            bf2_sb = cload(bf2_d, [128, 1])
            bf3_sb = cload(bf3_d, [1, 1])
            ifoldb_sb = cload(ifoldb_d, [128, D], bf16)
            ifold8_sb = cp.tile([128, 2, D], fp8, tag="Ifold8")
            dma(ifold8_sb[:], ifold8_d)
            maskL2_sb = cload(maskL2_d, [128, NT])
            maskD2_sb = cload(maskD2_d, [128, NT])

            # one PSUM arena, manually laid out
            F = ps.tile([128, 4096], f32, tag="F")

            # ---------------- protein conv (L-sharded) ----------------
            p_sb = []
            for m in range(2):
                pslice = F[:, m * 512:(m + 1) * 512]
                for i in range(8):
                    nc.tensor.matmul(
                        pslice, WcT_sb[i][:, m * 128:(m + 1) * 128],
                        protT_sb[i][:], start=(i == 0), stop=(i == 7))
                pt = wp.tile([128, L_SH], bf16, tag=f"p{m}")
                nc.scalar.activation(pt[:], pslice, AF.Relu, bias=bc_sb[m][:])
                p_sb.append(pt)

            psk = F[0:D, 1024:1536]
            nc.tensor.matmul(psk, W1T_sb[0][:], p_sb[0][:],
                             start=True, stop=False)
            nc.tensor.matmul(psk, W1T_sb[1][:], p_sb[1][:],
                             start=False, stop=True)
            # stack [k ; pa] for the AllGather (k on top: matmul rhs needs
            # base partition 0)
            ag_sb = wp.tile([128, L_SH], bf16, tag="ag")
            nc.scalar.activation(ag_sb[0:D, :], psk, AF.Relu, bias=b1_sb[:])

            pspa = F[0:D, 1536:2048]
            nc.tensor.matmul(pspa, WpaT_sb[:], ag_sb[0:D, :],
                             start=True, stop=True)
            # evac at base 0, then DMA into the upper half (ACT cannot
            # shift partitions; DMA can)
            patmp = wp.tile([D, L_SH], bf16, tag="patmp")
            nc.scalar.activation(patmp[:], pspa, AF.Identity, bias=bpa_sb[:])
            dma(ag_sb[D:128, :], patmp[:])

            # ---------------- AllGather pa/k ----------------
            cc_ag_in = dp.tile([128, L_SH], bf16)
            cc_ag_out = dp.tile([NCORES, 128, L_SH], bf16, addr_space="Shared")
            dma(cc_ag_in[:], ag_sb[:])
            nc.gpsimd.collective_compute(
                "AllGather", mybir.AluOpType.bypass,
                replica_groups=[list(range(NCORES))],
                ins=[cc_ag_in[:].opt()],
                outs=[cc_ag_out[:].opt()],
            )

            # ---------------- reaction side (local, overlaps AG) --------
            psr = F[0:D, 2048:2048 + Q_LOC]
            nc.tensor.matmul(psr, W2T_sb[0][:], reactT_sb[0][:],
                             start=True, stop=False)
            nc.tensor.matmul(psr, W2T_sb[1][:], reactT_sb[1][:],
                             start=False, stop=True)
            r_sb = wp.tile([D, Q_LOC], bf16, tag="r")
            nc.scalar.activation(r_sb[:], psr, AF.Relu, bias=b2_sb[:])

            psra = F[0:D, 2112:2112 + Q_LOC]
            nc.tensor.matmul(psra, WraT_sb[:], r_sb[:], start=True, stop=True)
            ra_sb = wp.tile([D, Q_LOC], f32, tag="ra")    # +ra
            nra_sb = wp.tile([D, Q_LOC], f32, tag="nra")  # -ra
            nc.scalar.activation(ra_sb[:], psra, AF.Identity, bias=bra_sb[:])
            nc.scalar.activation(nra_sb[:], psra, AF.Identity,
                                 bias=nbra_sb[:], scale=-1.0)

            # q-pair layout: col j <-> q=j (rows 0:64), q=j+32 (rows 64:128)
            ra2 = wp.tile([128, NT], f32, tag="ra2")
            nra2 = wp.tile([128, NT], f32, tag="nra2")
            dma(ra2[0:D, :], ra_sb[:, 0:NT])
            dma(ra2[D:128, :], ra_sb[:, NT:Q_LOC])
            dma(nra2[0:D, :], nra_sb[:, 0:NT])
            dma(nra2[D:128, :], nra_sb[:, NT:Q_LOC])

            # pa2: pa duplicated on both partition halves; k_full separate
            # (AG output is [core, chan, x]; transpose to [chan, core, x])
            pa2 = wp.tile([128, L], bf16, tag="pa2")
            k_sb = wp.tile([D, L], bf16, tag="k")
            ag_k = cc_ag_out[:, 0:D, :].transpose([1, 0, 2])
            ag_pa = cc_ag_out[:, D:128, :].transpose([1, 0, 2])
            dma(pa2[0:D, :], ag_pa)
            dma(pa2[D:128, :], ag_pa)
            dma(k_sb[:], ag_k)

            # ---------------- pairwise stage ----------------
            acc2 = wp.tile([128, NT], f32, tag="acc2")

            def fold(rhs_ap, first, last, dr):
                for ch in range(NCH):
                    if dr:
                        nc.tensor.matmul(
                            F[0:D, ch * 512:(ch + 1) * 512],
                            ifold8_sb[:, :, :],
                            rhs_ap[:, :, ch * 512:(ch + 1) * 512],
                            start=first, stop=last, perf_mode=DR)
                    else:
                        nc.tensor.matmul(
                            F[0:D, ch * 512:(ch + 1) * 512],
                            ifoldb_sb[:],
                            rhs_ap[:, ch * 512:(ch + 1) * 512],
                            start=first, stop=last)

            nunits = len(units)
            for ui, (j0, w, t) in enumerate(units):
                first, last = (ui == 0), (ui == nunits - 1)
                if t == "B":
                    tmp = tpb.tile([128, L], bf16, tag="tb")
                    nc.vector.tensor_scalar(
                        tmp[:], pa2[:], nra2[:, j0:j0 + 1], ra2[:, j0:j0 + 1],
                        ALU.max, ALU.add, accum_out=acc2[:, j0:j0 + 1])
                    fold(tmp, first, last, dr=False)
                elif t == "8":
                    pair = tp8.tile([128, 2, L], fp8, tag="t8")
                    for s in range(2):
                        j = j0 + s
                        nc.vector.tensor_scalar(
                            pair[:, s, :], pa2[:], nra2[:, j:j + 1],
                            ra2[:, j:j + 1], ALU.max, ALU.add,
                            accum_out=acc2[:, j:j + 1])
                    fold(pair, first, last, dr=True)
                else:  # 'A'
                    pair = tp8.tile([128, 2, L], fp8, tag="tA")
                    for s in range(2):
                        j = j0 + s
                        nc.scalar.activation(
                            pair[:, s, :], pa2[:], AF.Relu,
                            bias=ra2[:, j:j + 1],
                            accum_out=acc2[:, j:j + 1])
                    fold(pair, first, last, dr=True)

            # masked ra column-sum for the S_p correction (rides the AR)
            rad = wp.tile([128, NT], f32, tag="rad")
            nc.vector.tensor_tensor(rad[:], ra2[:], maskD2_sb[:], op=ALU.mult)
            sumra2 = wp.tile([128, 1], f32, tag="sumra2")
            nc.vector.reduce_sum(sumra2[:], rad[:], axis=mybir.AxisListType.X)

            # ---------------- F evac + AllReduce ----------------
            # NOTE: the whole F[0:64, :] region belongs to the fold until
            # these evacs have read it; all later psum users come after.
            fsb = wp.tile([128, 2048 + 1], bf16, tag="fsb")
            nc.scalar.activation(fsb[0:D, 0:2048], F[0:D, 0:2048], AF.Copy)
            # DVE 64-part op may shift partition halves (quadrant-aligned)
            nc.vector.tensor_scalar(fsb[D:128, 0:2048], F[0:D, 2048:4096],
                                    0.0, None, ALU.add)
            nc.vector.tensor_scalar(fsb[:, 2048:2049], sumra2[:], 0.0, None,
                                    ALU.add)

            cc_ar_in = dp.tile([128, 2049], bf16)
            cc_ar_out = dp.tile([128, 2049], bf16, addr_space="Shared")
            dma(cc_ar_in[:], fsb[:])
            nc.gpsimd.collective_compute(
                "AllReduce", mybir.AluOpType.add,
                replica_groups=[list(range(NCORES))],
                ins=[cc_ar_in[:].opt()],
                outs=[cc_ar_out[:].opt()],
            )

            # ---------------- S_r (local) -> rxnfp; overlaps the AR ------
            corr = wp.tile([128, NT], f32, tag="corr")
            nc.vector.tensor_tensor(corr[:], ra2[:], maskL2_sb[:],
                                    op=ALU.mult)
            sr2 = wp.tile([128, NT], bf16, tag="sr2")
            nc.vector.tensor_tensor(sr2[:], acc2[:], corr[:], op=ALU.add)
            sr = wp.tile([D, Q_LOC], bf16, tag="sr")
            dma(sr[:, 0:NT], sr2[0:D, :])
            dma(sr[:, NT:Q_LOC], sr2[D:128, :])

            psrg = F[0:D, 2176:2176 + Q_LOC]
            nc.tensor.matmul(psrg, WaT_sb[:], sr[:], start=True, stop=True)
            rgate = wp.tile([D, Q_LOC], f32, tag="rgate")
            nc.scalar.activation(rgate[:], psrg, AF.Sigmoid, bias=ba_sb[:],
                                 scale=1.0 / L)
            rx = wp.tile([D, Q_LOC], bf16, tag="rx")
            nc.vector.scalar_tensor_tensor(rx[:], rgate[:], 1.0, r_sb[:],
                                           op0=ALU.add, op1=ALU.mult)

            fsum = wp.tile([D, L], bf16, tag="fsum")
            dma(fsum[:, 0:2048], cc_ar_out[0:D, 0:2048])
            dma(fsum[:, 2048:4096], cc_ar_out[D:128, 0:2048])
            sra_t = wp.tile([D, 1], bf16, tag="sra_t")
            sra_b = wp.tile([D, 1], bf16, tag="sra_b")
            dma(sra_t[:], cc_ar_out[0:D, 2048:2049])
            dma(sra_b[:], cc_ar_out[D:128, 2048:2049])
            sumra_g = wp.tile([D, 1], bf16, tag="sumra_g")
            nc.vector.tensor_tensor(sumra_g[:], sra_t[:], sra_b[:],
                                    op=ALU.add)

            # p_gate bias col: ba + Wa @ sumra_glob / Q
            psbb = F[0:D, 2304:2305]
            nc.tensor.matmul(psbb, WaT_sb[:], sumra_g[:],
                             start=True, stop=True)
            babar = wp.tile([D, 1], f32, tag="babar")
            nc.scalar.activation(babar[:], psbb, AF.Identity, bias=ba_sb[:],
                                 scale=1.0 / Q)

            # ---------------- p_gate / prot (replicated) ----------------
            # chunked pipeline: MM -> sigmoid -> k*pg -> g=(k+kp), max-accum
            USE_TTR = os.environ.get("K_TTR", "0") == "1"
            pgate = wp.tile([D, L], bf16, tag="pgate")
            kp_sb = wp.tile([D, L], bf16, tag="kp")
            g_sb = wp.tile([D, L], bf16, tag="g")
            protm = wp.tile([D, NCH], f32, tag="protm")
            for half in range(2):
                sl = slice(half * 2048, (half + 1) * 2048)
                for ch in range(4 * half, 4 * half + 4):
                    pspg = F[0:D, ch * 512:(ch + 1) * 512]
                    nc.tensor.matmul(pspg, WaT_sb[:],
                                     fsum[:, ch * 512:(ch + 1) * 512],
                                     start=True, stop=True)
                    nc.scalar.activation(
                        pgate[:, ch * 512:(ch + 1) * 512], pspg,
                        AF.Sigmoid, bias=babar[:], scale=1.0 / Q)
                nc.vector.tensor_tensor(kp_sb[:, sl], k_sb[:, sl],
                                        pgate[:, sl], op=ALU.mult)
                if USE_TTR:
                    for ch in range(4 * half, 4 * half + 4):
                        csl = slice(ch * 512, (ch + 1) * 512)
                        nc.vector.tensor_tensor_reduce(
                            g_sb[:, csl], k_sb[:, csl], kp_sb[:, csl],
                            1.0, 0.0, ALU.add, ALU.max,
                            accum_out=protm[:, ch:ch + 1])
                else:
                    nc.vector.tensor_tensor(g_sb[:, sl], k_sb[:, sl],
                                            kp_sb[:, sl], op=ALU.add)
                    nc.vector.reduce_max(protm[:, half:half + 1],
                                         g_sb[:, sl],
                                         axis=mybir.AxisListType.X)
            protg = wp.tile([D, 1], bf16, tag="protg")
            nc.vector.reduce_max(protg[:], protm[:, 0:2],
                                 axis=mybir.AxisListType.X)

            # ---------------- MLP head (local q) ----------------
            h1_sb = []
            for m in range(2):
                pst = F[0:128, 2432 + m * 2:2432 + m * 2 + 1]
                nc.tensor.matmul(pst, Wf1bT_sb[:, m * 128:(m + 1) * 128],
                                 protg[:], start=True, stop=True)
                fold_sb = wp.tile([128, 1], f32, tag=f"fold{m}")
                nc.scalar.activation(fold_sb[:], pst, AF.Identity,
                                     bias=bf1_sb[m][:])
                psh = F[:, 2560 + m * 64:2560 + (m + 1) * 64]
                nc.tensor.matmul(psh, Wf1aT_sb[:, m * 128:(m + 1) * 128],
                                 rx[:], start=True, stop=True)
                h1l = wp.tile([128, Q_LOC], f32, tag=f"h1l{m}")
                nc.scalar.activation(h1l[:], psh, AF.Identity,
                                     bias=fold_sb[:])
                h1 = wp.tile([128, Q_LOC], bf16, tag=f"h1{m}")
                nc.vector.scalar_tensor_tensor(h1[:], h1l[:], 0.01, h1l[:],
                                               op0=ALU.mult, op1=ALU.max)
                h1_sb.append(h1)

            psh2 = F[:, 2816:2816 + Q_LOC]
            nc.tensor.matmul(psh2, Wf2T_sb[0][:], h1_sb[0][:],
                             start=True, stop=False)
            nc.tensor.matmul(psh2, Wf2T_sb[1][:], h1_sb[1][:],
                             start=False, stop=True)
            h2l = wp.tile([128, Q_LOC], f32, tag="h2l")
            nc.scalar.activation(h2l[:], psh2, AF.Identity, bias=bf2_sb[:])
            h2 = wp.tile([128, Q_LOC], bf16, tag="h2")
            nc.vector.scalar_tensor_tensor(h2[:], h2l[:], 0.01, h2l[:],
                                           op0=ALU.mult, op1=ALU.max)

            if DEBUG:
                dbgfs = wp.tile([128, 2049], f32, tag="dbgfs")
                nc.vector.tensor_scalar(dbgfs[:], fsb[:], 0.0, None, ALU.add)
                dma(dbg_fsb, dbgfs[:])
                dbgp = wp.tile([D, 1], f32, tag="dbgp")
                nc.vector.tensor_scalar(dbgp[:], protg[:], 0.0, None, ALU.add)
                dma(dbg_prot, dbgp[:])
                dbgf = wp.tile([D, L], f32, tag="dbgf")
                nc.vector.tensor_scalar(dbgf[:], fsum[:], 0.0, None, ALU.add)
                dma(dbg_fsum, dbgf[:])
                dbgs = wp.tile([D, Q_LOC], f32, tag="dbgs")
                nc.vector.tensor_scalar(dbgs[:], sr[:], 0.0, None, ALU.add)
                dma(dbg_sr, dbgs[:])
                dbgg = wp.tile([D, L], f32, tag="dbgg")
                nc.vector.tensor_scalar(dbgg[:], pgate[:], 0.0, None, ALU.add)
                dma(dbg_pg, dbgg[:])
                dbgsr = wp.tile([D, 1], f32, tag="dbgsr")
                nc.vector.tensor_scalar(dbgsr[:], sumra_g[:], 0.0, None,
                                        ALU.add)
                dma(dbg_sumra, dbgsr[:])

            pso = F[0:1, 3072:3072 + Q_LOC]
            nc.tensor.matmul(pso, Wf3T_sb[:], h2[:], start=True, stop=True)
            out_sb = wp.tile([1, Q_LOC], f32, tag="out")
            nc.scalar.activation(out_sb[:], pso, AF.Identity, bias=bf3_sb[:])
            dma(out_d, out_sb[:])

    nc.compile()
    return nc


def _get_nc():
    key = (SCHED,)
    if key not in _CACHE:
        _CACHE[key] = _build()
    return _CACHE[key]


def _prep_in_maps(inputs):
    from concourse import mybir
    bf16_np = mybir.dt.np(mybir.dt.bfloat16)
    fp8_np = mybir.dt.np(mybir.dt.float8e4)

    f = lambda x: np.ascontiguousarray(np.asarray(x), dtype=np.float32)
    protein = f(inputs["protein"])[0]          # [L, 1024]
    reactions = f(inputs["reactions"])[0]      # [Q, 256]
    Wc, bc = f(inputs["Wc"]), f(inputs["bc"])
    W1, b1 = f(inputs["W1"]), f(inputs["b1"])
    W2, b2 = f(inputs["W2"]), f(inputs["b2"])
    Wa, ba = f(inputs["Wa"]), f(inputs["ba"])
    Wpa, bpa = f(inputs["Wpa"]), f(inputs["bpa"])
    Wra, bra = f(inputs["Wra"]), f(inputs["bra"])
    Wf1, bf1 = f(inputs["Wf1"]), f(inputs["bf1"])
    Wf2, bf2 = f(inputs["Wf2"]), f(inputs["bf2"])
    Wf3, bf3 = f(inputs["Wf3"]), f(inputs["bf3"])

    c = np.ascontiguousarray
    eye = np.eye(D, dtype=np.float32)
    ifoldb = np.concatenate([eye, eye], axis=0)           # [128, 64]
    ifold8 = np.concatenate([ifoldb, ifoldb], axis=1)     # [128, 128]

    is_dve = np.array([1.0 if t in "B8" else 0.0 for t in SCHED],
                      np.float32)
    maskL2 = np.tile(is_dve * (L - 1), (128, 1)).astype(np.float32)
    maskD2 = np.tile(is_dve, (128, 1)).astype(np.float32)

    common = {
        "WcT": c(Wc.T).astype(bf16_np),
        "W1T": c(W1.T).astype(bf16_np),
        "W2T": c(W2.T).astype(bf16_np),
        "WaT": c(Wa.T).astype(bf16_np),
        "WpaT": c(Wpa.T).astype(bf16_np),
        "WraT": c(Wra.T).astype(bf16_np),
        "Wf1aT": c(Wf1[:, :D].T).astype(bf16_np),
        "Wf1bT": c(Wf1[:, D:].T).astype(bf16_np),
        "Wf2T": c(Wf2.T).astype(bf16_np),
        "Wf3T": c(Wf3.T).astype(bf16_np),
        "bc": bc.reshape(-1, 1),
        "b1": b1.reshape(-1, 1),
        "b2": b2.reshape(-1, 1),
        "ba": ba.reshape(-1, 1),
        "bpa": bpa.reshape(-1, 1),
        "bra": bra.reshape(-1, 1),
        "nbra": (-bra).reshape(-1, 1),
        "bf1": bf1.reshape(-1, 1),
        "bf2": bf2.reshape(-1, 1),
        "bf3": bf3.reshape(-1, 1),
        "Ifoldb": ifoldb.astype(bf16_np),
        "Ifold8": ifold8.astype(fp8_np),
        "maskL2": maskL2,
        "maskD2": maskD2,
    }
    reactT = c(reactions.T).astype(bf16_np)    # [256, 512]
    in_maps = []
    for d in range(NCORES):
        shard = c(protein[d * L_SH:(d + 1) * L_SH, :].T).astype(bf16_np)
        in_maps.append({**common, "protT": shard,
                        "reactT": c(reactT[:, d * Q_LOC:(d + 1) * Q_LOC])})
    return in_maps


def run(inputs, trace=False, **kw):
    from concourse import bass_utils
    nc = _get_nc()
    in_maps = _prep_in_maps(inputs)
    res = bass_utils.run_bass_kernel_spmd(
        nc, in_maps, core_ids=list(range(NCORES)), trace=trace, **kw)
    return res


def kernel(**inputs):
    res = run(inputs)
    return np.concatenate(
        [np.asarray(res.results[d]["out"], np.float32).reshape(-1)
         for d in range(NCORES)])
